# revision 1
# baseline (speedup 1.0000x reference)
"""Minibatch discrimination (Salimans et al. 2016) on 8 Trainium2 cores.

Reference computation:
    m = (x @ W).reshape(B, K, D)                      # [1024, 32, 8]
    L1[b1, k, b2] = sum_d |m[b1,k,d] - m[b2,k,d]|
    mb[b1, k]     = sum_b2 exp(-L1[b1, k, b2])
    out           = concat([x, mb], axis=-1)          # [1024, 2080]

Sharding: data-parallel over batch rows (128 rows/core). The [256, 128]
per-core m^T shard is AllGathered (bf16) so every core holds m^T for all
1024 rows, then each core computes its 128-row block of the pairwise
reduction.

Per-core microkernel layout: kd=K*D=256 on partitions (2 chunks of 128),
b2=1024 on the free dim. Per query row i:
  - DVE dual-op tensor_scalar: |M_T[kd, b2] - m_T_local[kd, i]| in ONE
    bf16 pass per chunk (subtract then abs_max with 0).
  - PE matmul vs a block-diagonal ones matrix sums over d (partition
    reduction kd -> k) into PSUM, 4 query rows packed per [128, 1024]
    PSUM tile.
  - ACT Exp(scale=-1) with fused accum_out produces sum_b2 exp(-L1) in
    one pass per 4 rows.
"""

import sys

sys.path.insert(0, "/opt/trn_rl_repo")

import numpy as np
import ml_dtypes

import concourse.bass as bass
import concourse.bacc as bacc
import concourse.mybir as mybir
import concourse.tile as tile
from concourse.bass_utils import run_bass_kernel_spmd

B, F = 1024, 2048
NK, KDIM = 32, 8
KD = NK * KDIM  # 256
NCORES = 8
RB = B // NCORES  # 128 rows per core
FOUT = F + NK  # 2080


def emit_kernel(nc, tc, x_ap, w_ap, out_ap):
    f32 = mybir.dt.float32
    bf16 = mybir.dt.bfloat16
    AF = mybir.ActivationFunctionType
    ALU = mybir.AluOpType

    # Constants embedded in the NEFF.
    ident_np = np.eye(128, dtype=np.float32)
    l0 = np.zeros((128, NK), np.float32)
    l0[np.arange(128), np.arange(128) // KDIM] = 1.0  # kd 0..127 -> k 0..15
    l1 = np.zeros((128, NK), np.float32)
    l1[np.arange(128), 16 + np.arange(128) // KDIM] = 1.0  # kd 128..255 -> k 16..31
    ident_d = nc.inline_tensor(ident_np, name="ident_c")
    lhs0_d = nc.inline_tensor(l0.astype(ml_dtypes.bfloat16), name="lhs0_c")
    lhs1_d = nc.inline_tensor(l1.astype(ml_dtypes.bfloat16), name="lhs1_c")
    lhs1n_d = nc.inline_tensor((-l1).astype(ml_dtypes.bfloat16), name="lhs1n_c")
    lhs1x2_d = nc.inline_tensor((2 * l1).astype(ml_dtypes.bfloat16), name="lhs1x2_c")

    with (
        tc.tile_pool(name="persist", bufs=1) as pp,
        tc.tile_pool(name="dram", bufs=1, space="DRAM") as dp,
    ):
        ident = pp.tile([128, 128], f32, name="ident")
        nc.sync.dma_start(ident[:], ident_d.ap())
        identb = pp.tile([128, 128], bf16, name="identb")
        nc.vector.tensor_copy(identb[:], ident[:])
        lhs0 = pp.tile([128, NK], bf16, name="lhs0")
        nc.sync.dma_start(lhs0[:], lhs0_d.ap())
        lhs1 = pp.tile([128, NK], bf16, name="lhs1")
        nc.sync.dma_start(lhs1[:], lhs1_d.ap())
        lhs1n = pp.tile([128, NK], bf16, name="lhs1n")
        nc.sync.dma_start(lhs1n[:], lhs1n_d.ap())
        lhs1x2 = pp.tile([128, NK], bf16, name="lhs1x2")
        nc.sync.dma_start(lhs1x2[:], lhs1x2_d.ap())

        # m^T of the local shard, bf16: chunk0 = kd 0..127, chunk1 = kd 128..255
        mTloc0 = pp.tile([128, RB], bf16, name="mTloc0")
        mTloc1 = pp.tile([128, RB], bf16, name="mTloc1")
        # f32 copies of the bf16-rounded values (tensor_scalar needs f32
        # scalars; round-tripping keeps the self-pair diff exactly zero).
        mTloc0f = pp.tile([128, RB], f32, name="mTloc0f")
        mTloc1f = pp.tile([128, RB], f32, name="mTloc1f")
        mTloc1n = pp.tile([128, RB], f32, name="mTloc1n")  # negated, ACT bias
        # m^T of ALL rows (post-gather), bf16
        MT0 = pp.tile([128, B], bf16, name="MT0")
        MT1 = pp.tile([128, B], bf16, name="MT1")
        # mb accumulator: row p = 32*(i%4) + k, col g = i//4  (i = query row)
        mbcols = pp.tile([128, NK], f32, name="mbcols")

        cc_in = dp.tile([KD, RB], bf16, name="cc_in")
        mgat = dp.tile([NCORES * KD, RB], bf16, addr_space="Shared", name="mgat")

        # ---------------- Stage A: m^T = (x_shard @ W)^T, gather ----------
        with (
            tc.tile_pool(name="sa", bufs=16) as sa,
            tc.tile_pool(name="sa1", bufs=1) as sa1,
            tc.tile_pool(name="pps", bufs=2, space="PSUM") as ps,
            tc.tile_pool(name="mps", bufs=1, space="PSUM") as mps,
        ):
            xsb = sa1.tile([128, F], f32, name="xsb")
            nc.sync.dma_start(xsb[:], x_ap[:, :])
            # bf16 everywhere in the m matmul: full-rate PE (1 cyc/row vs 4
            # for f32); the bf16 rounding of m matches the bf16 M_T used in
            # stage B, so self-pairs still give exactly L1=0.
            xsbb = sa1.tile([128, F], bf16, name="xsbb")
            nc.vector.tensor_copy(xsbb[:], xsb[:])

            xts = []
            for i in range(16):
                pt = ps.tile([128, 128], bf16, name="pt")
                nc.tensor.transpose(pt[:], xsbb[:, i * 128 : (i + 1) * 128], identb[:])
                xt = sa.tile([128, 128], bf16, name="xt")
                nc.vector.tensor_copy(xt[:], pt[:])
                xts.append(xt)

            m0 = mps.tile([128, RB], f32, name="m0")
            m1 = mps.tile([128, RB], f32, name="m1")
            wsbs = []
            for i in range(16):
                wsb = sa.tile([128, KD], f32, name="wsb")
                nc.gpsimd.dma_start(wsb[:], w_ap[i * 128 : (i + 1) * 128, :])
                wsbb = sa.tile([128, KD], bf16, name="wsbb")
                nc.vector.tensor_copy(wsbb[:], wsb[:])
                wsbs.append(wsbb)
            for i in range(16):
                nc.tensor.matmul(
                    m0[:], wsbs[i][:, 0:128], xts[i][:],
                    start=(i == 0), stop=(i == 15),
                )
            for i in range(16):
                nc.tensor.matmul(
                    m1[:], wsbs[i][:, 128:KD], xts[i][:],
                    start=(i == 0), stop=(i == 15),
                )

            nc.vector.tensor_copy(mTloc0[:], m0[:])  # f32 PSUM -> bf16 SBUF
            nc.vector.tensor_copy(mTloc1[:], m1[:])
            nc.vector.tensor_copy(mTloc0f[:], mTloc0[:])  # bf16 -> f32 exact
            nc.vector.tensor_copy(mTloc1f[:], mTloc1[:])
            nc.vector.tensor_scalar(
                mTloc1n[:], mTloc1f[:], -1.0, None, ALU.mult
            )

        nc.sync.dma_start(cc_in[0:128, :], mTloc0[:])
        nc.sync.dma_start(cc_in[128:KD, :], mTloc1[:])
        nc.gpsimd.collective_compute(
            "AllGather",
            mybir.AluOpType.bypass,
            replica_groups=[list(range(NCORES))],
            ins=[cc_in.opt()],
            outs=[mgat.opt()],
        )
        # One strided DMA per chunk: MT_j[kd, c*128 + b] = mgat[c*256 + j*128 + kd, b]
        mgv = mgat[:].rearrange("(c j p) b -> j p c b", c=NCORES, j=2)
        nc.sync.dma_start(MT0[:].rearrange("p (c b) -> p c b", c=NCORES), mgv[0])
        nc.gpsimd.dma_start(MT1[:].rearrange("p (c b) -> p c b", c=NCORES), mgv[1])

        # ---------------- Stage B: pairwise L1 -> exp -> sum --------------
        # L1[b1,k,b2] = sum_d |a_d - s_d| = SA[k,b2] + SS[k,b1] - 2*sum_d
        # min(a_d, s_d)   (a = M_T column b2, s = local query column b1).
        # The min term is one dual-op DVE pass per chunk (min, then *-2);
        # SA is PE-precomputed once and ACT-copied into PSUM per group; SS
        # folds into the exp bias.
        with tc.tile_pool(name="pre", bufs=1) as pre:
            # SA[k, b2] = sum_{d in k} M_T[kd, b2], replicated x4 vertically.
            # Stored as float32r (and fed through an f32r identity matmul)
            # so PE can inject it into PSUM at full rate with start=True.
            f32r = mybir.dt.float32r
            SA4 = pre.tile([128, B], f32r, name="SA4")
            identr = pre.tile([128, 128], f32r, name="identr")
            nc.vector.tensor_copy(identr[:], ident[:])
            SS4n = pre.tile([128, NK], f32, name="SS4n")
            with tc.tile_pool(name="prep", bufs=1, space="PSUM") as prep:
                # Chunk 0 uses |a-s| = a + s - 2*min(a,s); chunk 1 uses
                # |a-s| = (s-a) + 2*relu(a-s), so SA4 = SA_c0 - SA_c1.
                saps = prep.tile([32, B], f32, name="saps")
                for h in range(2):
                    sl = slice(h * 512, (h + 1) * 512)
                    nc.tensor.matmul(
                        saps[:, sl], lhs0[:], MT0[:, sl], start=True, stop=False
                    )
                    nc.tensor.matmul(
                        saps[:, sl], lhs1n[:], MT1[:, sl], start=False, stop=True
                    )
                for j in range(4):
                    nc.vector.tensor_copy(SA4[32 * j : 32 * j + 32, :], saps[:])
                # SS4n[32j+k, g] = -SS[k, 4g+j] = -sum_{d in k} mTloc[kd, 4g+j]
                ssps = prep.tile([32, RB], f32, name="ssps")
                nc.tensor.matmul(
                    ssps[:], lhs0[:], mTloc0[:], start=True, stop=False
                )
                nc.tensor.matmul(
                    ssps[:], lhs1[:], mTloc1[:], start=False, stop=True
                )
                ssn = pre.tile([32, RB], f32, name="ssn")
                nc.vector.tensor_scalar(ssn[:], ssps[:], -1.0, None, ALU.mult)
                # SS4n[32j + k, g] = ssn[k, 4g + j], one strided DMA per j
                ssn_v = ssn[:].rearrange("k (g j) -> k g j", j=4)
                for j in range(4):
                    nc.sync.dma_start(
                        SS4n[32 * j : 32 * j + 32, :], ssn_v[:, :, j]
                    )

            with (
                tc.tile_pool(name="ab", bufs=4) as ab,
                tc.tile_pool(name="pb", bufs=3, space="PSUM") as pb,
                tc.tile_pool(name="ep", bufs=2) as ep,
            ):
                for g in range(NK):
                    pg = pb.tile([128, B], f32, name="pg")
                    # Init PSUM with the SA term via a PE identity matmul
                    # (start=True sets has_written; a non-PE write would be
                    # overwritten by the first accumulating matmul).
                    for h in range(2):
                        sl = slice(h * 512, (h + 1) * 512)
                        nc.tensor.matmul(
                            pg[:, sl],
                            identr[:],
                            SA4[:, sl],
                            start=True, stop=False,
                            skip_group_check=True,
                        )
                    for j in range(4):
                        i = 4 * g + j
                        a0 = ab.tile([128, B], bf16, name="a0")
                        a1 = ab.tile([128, B], bf16, name="a1")
                        # chunk0: -2*min(a, s) in one dual-op DVE pass (GpSimd
                        # is ~38x slower for this; measured 15.5us/op).
                        nc.vector.tensor_scalar(
                            a0[:], MT0[:], mTloc0f[:, i : i + 1], -2.0,
                            ALU.min, ALU.mult,
                        )
                        # chunk1: relu(a - s); every 4th query on the ACT
                        # engine (which has slack) to relieve the DVE.
                        if i % 4 == 1 or i % 8 == 3:
                            nc.scalar.activation(
                                a1[:], MT1[:], AF.Relu,
                                bias=mTloc1n[:, i : i + 1], scale=1.0,
                            )
                        else:
                            nc.vector.tensor_scalar(
                                a1[:], MT1[:], mTloc1f[:, i : i + 1], 0.0,
                                ALU.subtract, ALU.max,
                            )
                        # c-major order: both halves of chunk 0, then chunk 1,
                        # so consecutive matmuls share stationary weights.
                        orows = slice(32 * j, 32 * j + 32)
                        for c, (lhsX, aX) in enumerate(((lhs0, a0), (lhs1x2, a1))):
                            for h in range(2):
                                sl = slice(h * 512, (h + 1) * 512)
                                nc.tensor.matmul(
                                    pg[orows, sl], lhsX[:], aX[:, sl],
                                    start=False, stop=(c == 1),
                                    tile_position=(0, 32 * j),
                                    skip_group_check=True,
                                )
                    es = ep.tile([128, B], bf16, name="es")
                    nc.scalar.activation(
                        es[:], pg[:], AF.Exp, scale=-1.0,
                        bias=SS4n[:, g : g + 1],
                        accum_out=mbcols[:, g : g + 1],
                    )

            # mbcols[32*j + k, g] holds mb for row i = 4*g + j, kernel k.
            with tc.tile_pool(name="finp", bufs=1, space="PSUM") as finp:
                mbT = finp.tile([NK, 128], f32, name="mbT")
                nc.tensor.transpose(mbT[:], mbcols[:], ident[:])
                mbTs = pre.tile([NK, 128], f32, name="mbTs")
                nc.vector.tensor_copy(mbTs[:], mbT[:])
                # out[4g + j, F + k] = mbTs[g, 32j + k]
                ov = out_ap[:, F:FOUT].rearrange("(g j) k -> g j k", j=4)
                nc.sync.dma_start(ov, mbTs[:].rearrange("g (j k) -> g j k", j=4))

        # Pass x through to out[:, :F] (independent; gpsimd queue keeps the
        # sync queue free for the latency-critical loads).
        nc.gpsimd.dma_start(out_ap[:, 0:F], x_ap[:, :])


def build_program():
    nc = bacc.Bacc("TRN2", num_devices=NCORES)
    x_sh = nc.dram_tensor("x_shard", [RB, F], mybir.dt.float32, kind="ExternalInput")
    w = nc.dram_tensor("W", [F, KD], mybir.dt.float32, kind="ExternalInput")
    out = nc.dram_tensor("out_shard", [RB, FOUT], mybir.dt.float32, kind="ExternalOutput")
    with tile.TileContext(nc, num_cores=NCORES) as tc:
        emit_kernel(nc, tc, x_sh.ap(), w.ap(), out.ap())
    nc.compile()
    return nc


def kernel(x, W):
    x = np.ascontiguousarray(np.asarray(x, dtype=np.float32))
    W = np.ascontiguousarray(np.asarray(W, dtype=np.float32))
    assert x.shape == (B, F) and W.shape == (F, KD)
    nc = build_program()
    in_maps = [
        {"x_shard": x[c * RB : (c + 1) * RB], "W": W} for c in range(NCORES)
    ]
    res = run_bass_kernel_spmd(nc, in_maps, core_ids=list(range(NCORES)))
    out = np.concatenate(
        [res.results[c]["out_shard"] for c in range(NCORES)], axis=0
    )
    return out.astype(np.float32)

